# revision 11
# baseline (speedup 1.0000x reference)
"""Binarized conv block (BinBlock) Trainium2 Bass kernel.

Reference computation (per image):
    xb    = sign(x)                                  # +/-1
    alpha = mean|W| over (I,kh,kw)                   # [O]
    wb    = alpha * sign(W)
    xp    = pad(xb, 1, value=-1)
    out   = conv2d(xp, wb) + bias
    out   = out*gBN + (beta - mean*gBN),  gBN = gamma/sqrt(var+eps)
    out   = out + x

Kernel algebra: let s = alpha*gBN, b2 = bias*gBN + beta - mean*gBN.
    out = s * conv2d(pad(sign(x),-1), sign(W)) + b2 + x
Activations binarize to a = 1[x>=0] in {0,1} (pad = 0), so with W' = sign(W):
    conv(a, W') = 0.5*conv_sign + 0.5*Wsum[o],  Wsum[o] = sum(sign(W[o]))
The residual is injected into the same PSUM accumulation through a
diag(1/(2s)) bf16 matmul on a bf16 copy of x (1 cycle/row; fp32 matmuls
are 4 cycles/row, and the fp32r/fp8 fast paths require dst partition 0 so
they cannot hit the upper PSUM half):
    psum = conv(a,W') + xb16/(2s)
    out  = psum*(2s) + (b2 - s*Wsum)     (single scalar-engine activation)

Sharding: batch 32 -> 4 images per core on 8 cores. Per core, images are
processed in pairs: image parity ih selects the SBUF partition half (input
row-group of the PE array); the PSUM partition half hf selects the image
row-half (rows 0:56 vs 56:112), i.e. step g convs block g and block 14+g.
That drives all four 64x64 PE array tiles concurrently with K=M=64 matmuls
AND makes each output-channel's staged rows contiguous in DRAM: the bf16
stage tile drains via HWDGE (sync queue) in 28-row descriptors (6272 B),
avoiding the small-packet SWDGE path that dominated earlier profiles.
Output is bf16 on device (upcast on host; rel err ~1e-3 vs the 2e-2 gate),
halving output HBM traffic.

Engine split: scalar casts x->bf16 (chunked) and runs the whole epilogue;
vector only binarizes (bf16 in/out -> 2x DVE mode) and memsets; the PE
carries conv+residual; sync issues every DMA. fp32 x lives only in small
transient chunk tiles. The next pair's cast/binarize is interleaved into
the g-loop so prefetch never head-of-line blocks the scalar queue.
"""

import numpy as np
import ml_dtypes

import concourse.bass as bass
import concourse.bacc as bacc
import concourse.tile as tile
import concourse.mybir as mybir
from concourse import bass_utils

F32 = mybir.dt.float32
BF16 = mybir.dt.bfloat16

B, C, H, W = 32, 64, 112, 112
NCORES = 8
BSH = B // NCORES          # images per core
HWF = H * W                # 12544
HP = H + 2                 # 114 padded
PADN = HP * HP             # 12996
NBLK = H // 4              # 28 four-row blocks
HB = NBLK // 2             # 14 blocks per image row-half (psum-half stream)
NB = 4 * W                 # 448 (fits one PSUM bank: 512 fp32)
GG = 7                     # g-steps per output drain group (28 rows)
CROWS = 16                 # input chunk rows
NCH = H // CROWS           # 7 chunks
BN_EPS = 1e-5

ACT_COPY = mybir.ActivationFunctionType.Copy
ACT_IDENT = mybir.ActivationFunctionType.Identity
OP_GE = mybir.AluOpType.is_ge


def build_kernel_body(tc, out_d, x_d, ws_d, wd_d, sb_d):
    nc = tc.nc
    with (
        tc.tile_pool(name="const", bufs=1) as constp,
        tc.tile_pool(name="xchunk", bufs=4) as xchp,
        tc.tile_pool(name="xbf", bufs=2) as xbfp,
        tc.tile_pool(name="sign", bufs=2) as signp,
        tc.tile_pool(name="stage", bufs=6) as stagep,
        tc.tile_pool(name="psum", bufs=8, space="PSUM") as psump,
    ):
        ws_t = constp.tile([128, 9 * C], BF16)   # sign(W)^T per position
        nc.sync.dma_start(ws_t[:], ws_d[:])
        wd_t = constp.tile([128, C], BF16)       # diag(1/(2s))
        nc.sync.dma_start(wd_t[:], wd_d[:])
        sb_t = constp.tile([128, 2], F32)        # col0: 2s, col1: b2 - s*Wsum
        nc.sync.dma_start(sb_t[:], sb_d[:])

        def pro_alloc(p):
            xb = xbfp.tile([128, HWF], BF16, name=f"xb_{p}", tag="xb")
            xb3 = xb[:].rearrange("p (h w) -> p h w", w=W)
            sg = signp.tile([128, PADN], BF16, name=f"sg_{p}", tag="sg")
            sg3 = sg[:].rearrange("p (h w) -> p h w", w=HP)
            # zero padding border ({0,1} convention: 0 == sign -1)
            nc.vector.memset(sg3[:, 0, :], 0.0)
            nc.vector.memset(sg3[:, HP - 1, :], 0.0)
            nc.vector.memset(sg3[:, 1 : HP - 1, 0], 0.0)
            nc.vector.memset(sg3[:, 1 : HP - 1, HP - 1], 0.0)
            return xb, xb3, sg3

        def pro_load(p, ci):
            ra = ci * CROWS
            xc = xchp.tile([128, CROWS * W], F32, name=f"xc_{p}_{ci}", tag="xc")
            nc.sync.dma_start(
                xc[:],
                x_d[2 * p : 2 * p + 2, :, ra : ra + CROWS, :].rearrange(
                    "b c h w -> (b c) (h w)"
                ),
            )
            return xc

        def pro_proc(p, xcs, xb3, sg3, ci):
            ra = ci * CROWS
            xc3 = xcs[ci][:].rearrange("p (h w) -> p h w", w=W)
            # scalar: fp32 -> bf16 copy of x (residual matmul operand)
            nc.scalar.activation(xb3[:, ra : ra + CROWS, :], xc3[:, :, :], ACT_COPY)
            # vector: binarize from the bf16 copy (2-byte in/out -> 2x DVE)
            nc.vector.tensor_scalar(
                sg3[:, 1 + ra : 1 + ra + CROWS, 1 : HP - 1],
                xb3[:, ra : ra + CROWS, :],
                0.0,
                None,
                OP_GE,
            )

        pro = {}

        def pro_start(p):
            xb, xb3, sg3 = pro_alloc(p)
            xcs = {ci: pro_load(p, ci) for ci in range(NCH)}
            pro[p] = (xb, xb3, sg3, xcs)

        pro_start(0)
        for ci in range(NCH):
            pro_proc(0, pro[0][3], pro[0][1], pro[0][2], ci)
        for p in range(BSH // 2):  # image pairs; image 2p -> partitions 0:64
            nxt = p + 1 if p + 1 < BSH // 2 else None
            if nxt is not None:
                pro_start(nxt)
            xb, _, sg3, _ = pro.pop(p)

            stages = [None, None]
            for g in range(HB):
                # interleave next pair's cast+binarize so scalar/vector work
                # for pair p is never queued behind a chunk that is still
                # loading
                if nxt is not None and g % 2 == 0 and g // 2 < NCH:
                    ci = g // 2
                    pro_proc(nxt, pro[nxt][3], pro[nxt][1], pro[nxt][2], ci)
                if g % GG == 0:
                    for ih in range(2):
                        stages[ih] = stagep.tile(
                            [128, GG * NB], BF16, name=f"st_p{p}g{g}i{ih}", tag="st"
                        )
                # One PSUM bank per image: partition half hf holds block
                # 14*hf+g (image row-half hf). start=True per slice makes the
                # first matmul an overwrite, so recycled banks need no clear.
                psb = [
                    psump.tile([128, NB], F32, name=f"ps_p{p}g{g}i{ih}", tag="ps")
                    for ih in range(2)
                ]
                # residual first: psum = diag(1/(2s)) @ xb16_block (bf16)
                for q in range(4):
                    ih, hf = divmod(q, 2)
                    blk = HB * hf + g
                    nc.tensor.matmul(
                        psb[ih][64 * hf : 64 * hf + 64, :],
                        wd_t[64 * ih : 64 * ih + 64, :],
                        xb[64 * ih : 64 * ih + 64, blk * NB : (blk + 1) * NB],
                        start=True,
                        stop=False,
                        skip_group_check=True,
                    )
                # 9 conv positions, round-robin over the 4 PE array tiles
                for pos in range(9):
                    dh, dw = divmod(pos, 3)
                    for q in range(4):
                        ih, hf = divmod(q, 2)
                        blk = HB * hf + g
                        r0 = 4 * blk + dh
                        nc.tensor.matmul(
                            psb[ih][64 * hf : 64 * hf + 64, :],
                            ws_t[64 * ih : 64 * ih + 64, 64 * pos : 64 * pos + 64],
                            sg3[64 * ih : 64 * ih + 64, r0 : r0 + 4, dw : dw + W],
                            start=False,
                            stop=(pos == 8),
                            skip_group_check=True,
                        )
                # epilogue: out = psum*(2s) + b2' in bf16, scalar engine only
                for ih in range(2):
                    nc.scalar.activation(
                        stages[ih][:, (g % GG) * NB : (g % GG + 1) * NB],
                        psb[ih][:, :],
                        ACT_IDENT,
                        bias=sb_t[:, 1:2],
                        scale=sb_t[:, 0:1],
                    )
                # drain: one HWDGE DMA per image per 28-row group; each
                # partition (hf,c) covers a contiguous 6272 B DRAM span
                if g % GG == GG - 1:
                    gg = g // GG
                    for ih in range(2):
                        n = 2 * p + ih
                        # dst iterates hf -> c -> span, matching the stage's
                        # partition order (hf*64+c); SBUF side must stay 2D
                        # (partition dim first), the DMA pairs elements by
                        # iteration order
                        dst = out_d[n].rearrange(
                            "c (hf hh) w -> hf c (hh w)", hf=2
                        )[:, :, gg * GG * NB : (gg + 1) * GG * NB]
                        nc.sync.dma_start(dst, stages[ih][:])


def build_nc():
    nc = bacc.Bacc(trn_type="TRN2", debug=False, num_devices=NCORES)
    x_d = nc.dram_tensor("x", [BSH, C, H, W], F32, kind="ExternalInput")
    ws_d = nc.dram_tensor("wsign", [128, 9 * C], BF16, kind="ExternalInput")
    wd_d = nc.dram_tensor("wdiag", [128, C], BF16, kind="ExternalInput")
    sb_d = nc.dram_tensor("scalebias", [128, 2], F32, kind="ExternalInput")
    out_d = nc.dram_tensor("out", [BSH, C, H, W], BF16, kind="ExternalOutput")
    with tile.TileContext(nc) as tc:
        build_kernel_body(tc, out_d, x_d, ws_d, wd_d, sb_d)
    nc.compile()
    return nc


def prep_consts(weight, bias, gamma, beta, run_mean, run_var):
    """Host-side constant prep (numpy, fp64 for the folding math)."""
    w = np.asarray(weight, np.float64)
    alpha = np.mean(np.abs(w), axis=(1, 2, 3))            # [O]
    g = np.asarray(gamma, np.float64) / np.sqrt(np.asarray(run_var, np.float64) + BN_EPS)
    s = alpha * g                                          # [O]
    b2 = np.asarray(bias, np.float64) * g + np.asarray(beta, np.float64) - np.asarray(
        run_mean, np.float64
    ) * g

    wsign = np.sign(w)                                     # [O,I,3,3]
    wsum = wsign.sum(axis=(1, 2, 3))                       # [O]
    # lhsT layout [I(dup to 128), pos, O]
    ws = wsign.transpose(1, 2, 3, 0).reshape(C, 9 * C)
    ws128 = np.concatenate([ws, ws], axis=0).astype(ml_dtypes.bfloat16)

    wd = np.zeros((C, C), np.float64)
    np.fill_diagonal(wd, 1.0 / (2.0 * s))
    wd128 = np.concatenate([wd, wd], axis=0).astype(ml_dtypes.bfloat16)
    # epilogue scale = 1/bf16(1/(2s)) exactly, so the residual coefficient
    # d*sc == 1 to fp32 precision (the conv term then carries the ~2^-9
    # bf16 rounding of d instead -- it is the smaller contributor). The
    # 0.5*Wsum*sc conv offset folds into the bias with the same sc.
    dinv = wd128.astype(np.float64).diagonal()[:C]         # bf16(1/(2s))
    sc64 = 1.0 / dinv
    b2 = b2 - 0.5 * sc64 * wsum
    sc = np.concatenate([sc64, sc64]).astype(np.float32)
    bi = np.concatenate([b2, b2]).astype(np.float32)
    sb128 = np.stack([sc, bi], axis=1)  # [128, 2]
    return ws128, wd128, sb128


_CACHE = {}


def kernel(x, weight, bias, gamma, beta, run_mean, run_var, _trace=False, _trace_kwargs=None):
    x = np.ascontiguousarray(np.asarray(x, np.float32))
    ws128, wd128, sb128 = prep_consts(weight, bias, gamma, beta, run_mean, run_var)

    if "nc" not in _CACHE:
        _CACHE["nc"] = build_nc()
    nc = _CACHE["nc"]

    in_maps = []
    for i in range(NCORES):
        in_maps.append(
            dict(
                x=x[BSH * i : BSH * (i + 1)],
                wsign=ws128,
                wdiag=wd128,
                scalebias=sb128,
            )
        )
    res = bass_utils.run_bass_kernel_spmd(
        nc,
        in_maps,
        core_ids=list(range(NCORES)),
        trace=_trace,
        **(_trace_kwargs or {}),
    )
    out = np.concatenate(
        [np.asarray(res.results[i]["out"], np.float32) for i in range(NCORES)],
        axis=0,
    )
    if _trace:
        kernel.last_results = res
    return out


# revision 14
# speedup vs baseline: 1.0549x; 1.0549x over previous
"""Binarized conv block (BinBlock) Trainium2 Bass kernel.

Reference computation (per image):
    xb    = sign(x)                                  # +/-1
    alpha = mean|W| over (I,kh,kw)                   # [O]
    wb    = alpha * sign(W)
    xp    = pad(xb, 1, value=-1)
    out   = conv2d(xp, wb) + bias
    out   = out*gBN + (beta - mean*gBN),  gBN = gamma/sqrt(var+eps)
    out   = out + x

Kernel algebra: let s = alpha*gBN, b2 = bias*gBN + beta - mean*gBN.
    out = s * conv2d(pad(sign(x),-1), sign(W)) + b2 + x
Activations binarize to a = 1[x>=0] in {0,1} fp8e4 (pad = 0); weights are
sign(W) in fp8e4, so every product is exact and
    conv(a, W') = 0.5*conv_sign + 0.5*Wsum[o].
The residual is injected into the same PSUM accumulation through a
diag(1/(2s)) bf16 matmul on a bf16 copy of x (1 cycle/row; fp32 matmuls
are 4 cycles/row, and the fp32r/fp8-DoubleRow fast paths require dst
partition 0 so they cannot reach the upper PSUM half):
    psum = conv(a,W') + xb16/(2s)
    out  = psum*(2s) + (b2 - s*Wsum)     (single scalar-engine activation)

Sharding: batch 32 -> 4 images per core on 8 cores. Per core, images are
processed in pairs: image parity ih selects the SBUF partition half (input
row-group of the PE array); the PSUM partition half hf selects the image
row-half (rows 0:56 vs 56:112), i.e. step g convs block g and block 14+g.
That drives all four 64x64 PE array tiles concurrently with K=M=64 matmuls
AND makes each output-channel's staged rows contiguous in DRAM.

DMA cost model learned from traces: an HWDGE dma_start burns ~27ns per
descriptor (= per partition, so ~3.5us per 128-partition DMA) on the
ISSUING engine as DMA_DIRECT2D, while SWDGE (gpsimd) triggers are cheap
but split transfers into ~1KB packets (~250GB/s aggregate ceiling). So:
bulk input (6 chunked loads) and output (4 whole-image stage drains,
GG=14) go HWDGE with the minimum DMA count, tiny consts go SWDGE, and the
final pair's two drains issue on different engines (sync+scalar) so their
descriptor generation overlaps. Output is bf16 (upcast on host; total rel
err ~2e-3 vs the 2e-2 gate), halving output HBM traffic.

Engine split: scalar does pair-0 x->bf16 casts + the whole epilogue;
vector binarizes (straight from the fp32 chunks) and casts pair 1; the PE
carries conv+residual; fp32 x lives only in two rotating chunk tiles.
"""

import numpy as np
import ml_dtypes

import concourse.bass as bass
import concourse.bacc as bacc
import concourse.tile as tile
import concourse.mybir as mybir
from concourse import bass_utils

F32 = mybir.dt.float32
BF16 = mybir.dt.bfloat16
FP8 = mybir.dt.float8e4

B, C, H, W = 32, 64, 112, 112
NCORES = 8
BSH = B // NCORES          # images per core
HWF = H * W                # 12544
HP = H + 2                 # 114 padded
PADN = HP * HP             # 12996
NBLK = H // 4              # 28 four-row blocks
HB = NBLK // 2             # 14 blocks per image row-half (psum-half stream)
NB = 4 * W                 # 448 (fits one PSUM bank: 512 fp32)
BN_EPS = 1e-5

ACT_IDENT = mybir.ActivationFunctionType.Identity
OP_GE = mybir.AluOpType.is_ge
OP_ADD = mybir.AluOpType.add

# input row chunks per pair: pair 0 finer for the lead-in, pair 1 coarse
# (it prefetches during pair 0's compute, issue cost matters more)
CHUNKS = {0: ((0, 32), (32, 64), (64, 88), (88, H)),
          1: ((0, 64), (64, H))}


def build_kernel_body(tc, out_d, x_d, ws_d, wd_d, sb_d):
    nc = tc.nc
    with (
        tc.tile_pool(name="const", bufs=1) as constp,
        tc.tile_pool(name="xchunk", bufs=2) as xchp,
        tc.tile_pool(name="xbf", bufs=2) as xbfp,
        tc.tile_pool(name="sign", bufs=2) as signp,
        tc.tile_pool(name="stage", bufs=4) as stagep,
        tc.tile_pool(name="psum", bufs=8, space="PSUM") as psump,
    ):
        # tiny consts via SWDGE: cheap trigger, no DIRECT2D burn on sync
        ws_t = constp.tile([128, 9 * C], FP8)    # sign(W)^T per position
        nc.gpsimd.dma_start(ws_t[:], ws_d[:])
        wd_t = constp.tile([128, C], BF16)       # diag(1/(2s))
        nc.gpsimd.dma_start(wd_t[:], wd_d[:])
        sb_t = constp.tile([128, 2], F32)        # col0: 2s', col1: b2'
        nc.gpsimd.dma_start(sb_t[:], sb_d[:])

        def pro_alloc(p):
            xb = xbfp.tile([128, HWF], BF16, name=f"xb_{p}", tag="xb")
            xb3 = xb[:].rearrange("p (h w) -> p h w", w=W)
            sg = signp.tile([128, PADN], FP8, name=f"sg_{p}", tag="sg")
            sg3 = sg[:].rearrange("p (h w) -> p h w", w=HP)
            # zero padding border ({0,1} convention: 0 == sign -1)
            nc.vector.memset(sg3[:, 0, :], 0.0)
            nc.vector.memset(sg3[:, HP - 1, :], 0.0)
            nc.vector.memset(sg3[:, 1 : HP - 1, 0], 0.0)
            nc.vector.memset(sg3[:, 1 : HP - 1, HP - 1], 0.0)
            return xb, xb3, sg3

        def pro_load(p, ci):
            ra, rb = CHUNKS[p][ci]
            xc = xchp.tile([128, (rb - ra) * W], F32, name=f"xc_{p}_{ci}", tag="xc")
            nc.sync.dma_start(
                xc[:],
                x_d[2 * p : 2 * p + 2, :, ra:rb, :].rearrange(
                    "b c h w -> (b c) (h w)"
                ),
            )
            return xc

        def pro_proc(p, xcs, xb3, sg3, ci):
            ra, rb = CHUNKS[p][ci]
            xc3 = xcs[ci][:].rearrange("p (h w) -> p h w", w=W)
            # x -> bf16 (residual matmul operand); same act func as the
            # epilogue so the scalar engine never reloads act tables.
            # pair 0's casts gate the lead-in -> scalar; pair 1's overlap
            # pair 0 compute -> vector (tensor_scalar add-0 as the cast).
            if p == 0:
                nc.scalar.activation(xb3[:, ra:rb, :], xc3[:, :, :], ACT_IDENT)
            else:
                nc.vector.tensor_scalar(
                    xb3[:, ra:rb, :], xc3[:, :, :], 0.0, None, OP_ADD
                )
            # binarize straight from the fp32 chunk (fp8 {0,1} out)
            nc.vector.tensor_scalar(
                sg3[:, 1 + ra : 1 + rb, 1 : HP - 1],
                xc3[:, :, :],
                0.0,
                None,
                OP_GE,
            )

        pro = {}

        def pro_start(p):
            xb, xb3, sg3 = pro_alloc(p)
            xcs = {ci: pro_load(p, ci) for ci in range(len(CHUNKS[p]))}
            pro[p] = (xb, xb3, sg3, xcs)

        pro_start(0)
        for ci in range(len(CHUNKS[0])):
            pro_proc(0, pro[0][3], pro[0][1], pro[0][2], ci)
        for p in range(BSH // 2):  # image pairs; image 2p -> partitions 0:64
            nxt = p + 1 if p + 1 < BSH // 2 else None
            if nxt is not None:
                pro_start(nxt)
            xb, _, sg3, _ = pro.pop(p)

            stages = [
                stagep.tile([128, HB * NB], BF16, name=f"st_p{p}i{ih}", tag="st")
                for ih in range(2)
            ]
            for g in range(HB):
                # interleave next pair's cast+binarize into the g-loop so
                # engine queues never head-of-line block on loads in flight
                if nxt is not None and g in (5, 10):
                    ci = 0 if g == 5 else 1
                    pro_proc(nxt, pro[nxt][3], pro[nxt][1], pro[nxt][2], ci)
                # One PSUM bank per image: partition half hf holds block
                # 14*hf+g (image row-half hf). start=True per slice makes the
                # first matmul an overwrite, so recycled banks need no clear.
                psb = [
                    psump.tile([128, NB], F32, name=f"ps_p{p}g{g}i{ih}", tag="ps")
                    for ih in range(2)
                ]
                # residual first: psum = diag(1/(2s)) @ xb16_block (bf16)
                for q in range(4):
                    ih, hf = divmod(q, 2)
                    blk = HB * hf + g
                    nc.tensor.matmul(
                        psb[ih][64 * hf : 64 * hf + 64, :],
                        wd_t[64 * ih : 64 * ih + 64, :],
                        xb[64 * ih : 64 * ih + 64, blk * NB : (blk + 1) * NB],
                        start=True,
                        stop=False,
                        skip_group_check=True,
                    )
                # 9 conv positions (fp8 {0,1} x {+-1}, exact), round-robin
                # over the 4 PE array tiles
                for pos in range(9):
                    dh, dw = divmod(pos, 3)
                    for q in range(4):
                        ih, hf = divmod(q, 2)
                        blk = HB * hf + g
                        r0 = 4 * blk + dh
                        nc.tensor.matmul(
                            psb[ih][64 * hf : 64 * hf + 64, :],
                            ws_t[64 * ih : 64 * ih + 64, 64 * pos : 64 * pos + 64],
                            sg3[64 * ih : 64 * ih + 64, r0 : r0 + 4, dw : dw + W],
                            start=False,
                            stop=(pos == 8),
                            skip_group_check=True,
                        )
                # epilogue: out = psum*(2s) + b2' in bf16, scalar engine only
                for ih in range(2):
                    nc.scalar.activation(
                        stages[ih][:, g * NB : (g + 1) * NB],
                        psb[ih][:, :],
                        ACT_IDENT,
                        bias=sb_t[:, 1:2],
                        scale=sb_t[:, 0:1],
                    )
            # drain: one HWDGE DMA per image covering its whole [hf][56-row]
            # stage; each partition (hf,c) is one contiguous 12544 B DRAM
            # span. The last pair splits across sync+scalar queues so the
            # two descriptor generations overlap in the tail.
            for ih in range(2):
                n = 2 * p + ih
                dst = out_d[n].rearrange("c (hf hh) w -> hf c (hh w)", hf=2)
                eng = nc.scalar if (nxt is None and ih == 0) else nc.sync
                eng.dma_start(dst, stages[ih][:])


def build_nc():
    nc = bacc.Bacc(trn_type="TRN2", debug=False, num_devices=NCORES)
    x_d = nc.dram_tensor("x", [BSH, C, H, W], F32, kind="ExternalInput")
    ws_d = nc.dram_tensor("wsign", [128, 9 * C], FP8, kind="ExternalInput")
    wd_d = nc.dram_tensor("wdiag", [128, C], BF16, kind="ExternalInput")
    sb_d = nc.dram_tensor("scalebias", [128, 2], F32, kind="ExternalInput")
    out_d = nc.dram_tensor("out", [BSH, C, H, W], BF16, kind="ExternalOutput")
    with tile.TileContext(nc) as tc:
        build_kernel_body(tc, out_d, x_d, ws_d, wd_d, sb_d)
    nc.compile()
    return nc


def prep_consts(weight, bias, gamma, beta, run_mean, run_var):
    """Host-side constant prep (numpy, fp64 for the folding math)."""
    w = np.asarray(weight, np.float64)
    alpha = np.mean(np.abs(w), axis=(1, 2, 3))            # [O]
    g = np.asarray(gamma, np.float64) / np.sqrt(np.asarray(run_var, np.float64) + BN_EPS)
    s = alpha * g                                          # [O]
    b2 = np.asarray(bias, np.float64) * g + np.asarray(beta, np.float64) - np.asarray(
        run_mean, np.float64
    ) * g

    wsign = np.sign(w)                                     # [O,I,3,3]
    wsum = wsign.sum(axis=(1, 2, 3))                       # [O]
    # lhsT layout [I(dup to 128), pos, O]; {+-1} exact in fp8e4
    ws = wsign.transpose(1, 2, 3, 0).reshape(C, 9 * C)
    ws128 = np.concatenate([ws, ws], axis=0).astype(ml_dtypes.float8_e4m3)

    wd = np.zeros((C, C), np.float64)
    np.fill_diagonal(wd, 1.0 / (2.0 * s))
    wd128 = np.concatenate([wd, wd], axis=0).astype(ml_dtypes.bfloat16)
    # epilogue scale = 1/bf16(1/(2s)) exactly, so the residual coefficient
    # d*sc == 1 to fp32 precision (the conv term then carries the ~2^-9
    # bf16 rounding of d instead -- it is the smaller contributor). The
    # 0.5*Wsum*sc conv offset folds into the bias with the same sc.
    dinv = wd128.astype(np.float64).diagonal()[:C]         # bf16(1/(2s))
    sc64 = 1.0 / dinv
    b2 = b2 - 0.5 * sc64 * wsum
    sc = np.concatenate([sc64, sc64]).astype(np.float32)
    bi = np.concatenate([b2, b2]).astype(np.float32)
    sb128 = np.stack([sc, bi], axis=1)  # [128, 2]
    return ws128, wd128, sb128


_CACHE = {}


def kernel(x, weight, bias, gamma, beta, run_mean, run_var, _trace=False, _trace_kwargs=None):
    x = np.ascontiguousarray(np.asarray(x, np.float32))
    ws128, wd128, sb128 = prep_consts(weight, bias, gamma, beta, run_mean, run_var)

    if "nc" not in _CACHE:
        _CACHE["nc"] = build_nc()
    nc = _CACHE["nc"]

    in_maps = []
    for i in range(NCORES):
        in_maps.append(
            dict(
                x=x[BSH * i : BSH * (i + 1)],
                wsign=ws128,
                wdiag=wd128,
                scalebias=sb128,
            )
        )
    res = bass_utils.run_bass_kernel_spmd(
        nc,
        in_maps,
        core_ids=list(range(NCORES)),
        trace=_trace,
        **(_trace_kwargs or {}),
    )
    out = np.concatenate(
        [np.asarray(res.results[i]["out"], np.float32) for i in range(NCORES)],
        axis=0,
    )
    if _trace:
        kernel.last_results = res
    return out
